# revision 17
# baseline (speedup 1.0000x reference)
"""Trainium2 Bass kernel for nn_GCLMemory (content-addressed memory read weights).

Full computation per batch sample b:
    dots[n]  = <keys[b,n,:], k[b,:]>
    cos[n]   = dots[n] / (max(||keys[b,n]||,eps) * max(||k[b]||,eps))
    wc       = softmax(beta[b] * cos)
    top-32 mask, renormalize, w = wc**gamma[b], renormalize.

Tail identity: the intermediate renormalizations cancel, so
    w = exp(gamma*logits) / sum  over the top-32 logits positions,
    logits = beta*cos.  Non-top leakage (1e-16 factor) is below fp32
    noise after **gamma, so it is dropped.

Sharding: data-parallel over batch. 8 cores x 16 samples.

Device-side layout (per core):
  - Host pre-transposes + casts keys to bf16: keysT [KQ=2, 128, 16, 2048]
    (K on partitions).  bf16 halves HBM traffic and runs the PE at
    1 cycle/row instead of fp32's 4.
  - lhsT kvT [KQ, 128, 32] bf16: col s = kvec of sample s, col 16 = ones.
    Per (sample, kind) job the PE streams 512-col slices of the raw
    (dots) or squared (sumsq) stream tile; useful output rows are s
    (dots) and 16 (sumsq).  4 jobs pack one [128, 2048] PSUM tile at
    partition bases 0/32/64/96.
  - PSUM tiles are bulk-copied to SBUF on ACT; per-job useful rows are
    extracted by DMA into D/S in a [128, 256] layout with partition
    p = 8*s + c (c = 256-col chunk of n), so the whole tail runs with
    128 active partitions (DVE/ACT cost scales with free size only).
  - Tail: rsb = exp(-0.5*ln(S) + ln(beta/||k||)); lg30 = D*rsb + 30
    (the +30 shift keeps all values positive for the match_replace
    top-k trick and is undone by the Exp bias); per-partition top-32
    via 4x max8 + match_replace; cross-chunk merge after a [128,32] ->
    [16,256] DMA reshape; threshold mask via is_ge; w = Exp(gamma*x -
    30*gamma) with accumulate; renorm via DMA-reshaped partial sums.
"""

import sys

import numpy as np

sys.path.insert(0, "/opt/trn_rl_repo")

import concourse.bass as bass
import concourse.mybir as mybir
from concourse.bass_utils import run_bass_kernel_spmd
from concourse.tile import TileContext

F32 = mybir.dt.float32
F32R = mybir.dt.float32r
BF16 = mybir.dt.bfloat16
Alu = mybir.AluOpType
Act = mybir.ActivationFunctionType
AxL = mybir.AxisListType

# ---------------------------------------------------------------------------
# This container's walrus build only accepts a single sem-wait command per
# instruction ("Too many sync wait commands" in CoreV3GenImpl otherwise), but
# Tile's exit drain aggregates one wait per busy processor. Split any
# multi-wait instruction into preceding single-wait Drains on the same engine.
# ---------------------------------------------------------------------------
_WAIT_LIMIT = 1


def _split_multi_waits(bir_bytes: bytes, limit: int = _WAIT_LIMIT) -> bytes:
    import orjson
    d = orjson.loads(bir_bytes)
    n_split = 0
    for fn in d.get("functions", []):
        for bb in fn.get("blocks", []):
            out = []
            for inst in bb.get("instructions", []):
                si = inst.get("sync_info") or {}
                waits = si.get("on_wait") or []
                if len(waits) > limit:
                    n_split += 1
                    chunks = [waits[i:i + limit]
                              for i in range(0, len(waits), limit)]
                    for j, ch in enumerate(chunks[:-1]):
                        carrier = {
                            "engine": inst["engine"],
                            "ins": [],
                            "is_reset_sema": False,
                            "name": f"{inst['name']}__w{j}",
                            "opcode": "Drain",
                            "outs": [],
                            "sync_info": {"on_update": [], "on_wait": ch},
                        }
                        if "debug" in inst:
                            carrier["debug"] = inst["debug"]
                        out.append(carrier)
                    si["on_wait"] = chunks[-1]
                out.append(inst)
            bb["instructions"] = out
    return orjson.dumps(d)


def _install_wait_split_hook():
    from concourse import bass2jax
    orig = bass2jax.compile_bir_kernel
    if getattr(orig, "_wait_split_wrapped", False):
        return

    def wrapped(bir_bytes, *args, **kwargs):
        return orig(_split_multi_waits(bir_bytes), *args, **kwargs)

    wrapped._wait_split_wrapped = True
    bass2jax.compile_bir_kernel = wrapped


_install_wait_split_hook()

B, N, K = 128, 2048, 256
M = 8            # cores
BPC = B // M     # samples per core
KQ = K // 128    # contraction chunks
NT = N // 512    # psum column tiles per sample
CAND = 32
GRP = 2          # samples per stream tile (16KB fp32 DMA lines)
NG = BPC // GRP  # stream groups
EPS = 1e-8
SHIFT = 30.0     # logits shift: keeps lg30 > 0 for the match_replace trick


def build_nc():
    nc = bass.Bass()
    keysT = nc.declare_dram_parameter("keysT", [KQ, 128, BPC, N], F32R,
                                      isOutput=False)
    kvTd = nc.declare_dram_parameter("kvTd", [128, BPC * KQ * 16], F32R,
                                     isOutput=False)
    onesT = nc.declare_dram_parameter("onesT", [128, BPC * 16], F32R,
                                      isOutput=False)
    lnsb = nc.declare_dram_parameter("lnsb", [128, 1], F32, isOutput=False)
    gam = nc.declare_dram_parameter("gam", [128, 1], F32, isOutput=False)
    gam16 = nc.declare_dram_parameter("gam16", [16, 1], F32, isOutput=False)
    ng30g16 = nc.declare_dram_parameter("ng30g16", [16, 1], F32,
                                        isOutput=False)
    out = nc.declare_dram_parameter("out", [128, N // 8], F32, isOutput=True)

    with TileContext(nc) as tc:
        with (
            tc.tile_pool(name="const", bufs=1) as cpool,
            tc.tile_pool(name="stream", bufs=3) as spool,
            tc.tile_pool(name="psum", bufs=1, space="PSUM") as ppool,
        ):
            # lhsT banks: per (sample s, q) a [128,32] fp32r tile with
            # kvec_s chunk q at col s (zeros elsewhere); per sample a ones
            # tile with 1.0 at col 16+s.  fp32r matmuls must write PSUM
            # partition base 0, so all 32 jobs accumulate into ONE shared
            # [32, 2048] PSUM tile: rows 0-15 collect dots (col s -> row s),
            # rows 16-31 collect sumsq; zero lhsT columns contribute 0.
            kvAll = cpool.tile([128, BPC * KQ * 16], F32R, tag="kvAll")
            nc.scalar.dma_start(out=kvAll[:], in_=kvTd[:])
            oneAll = cpool.tile([128, BPC * 16], F32R, tag="oneAll")
            nc.scalar.dma_start(out=oneAll[:], in_=onesT[:])
            lnsb_t = cpool.tile([128, 1], F32, tag="lnsb")
            nc.scalar.dma_start(out=lnsb_t[:], in_=lnsb[:])
            gam_t = cpool.tile([128, 1], F32, tag="gam")
            nc.scalar.dma_start(out=gam_t[:], in_=gam[:])
            gam16_t = cpool.tile([16, 1], F32, tag="gam16")
            nc.scalar.dma_start(out=gam16_t[:], in_=gam16[:])
            ng30g16_t = cpool.tile([16, 1], F32, tag="ng30g16")
            nc.scalar.dma_start(out=ng30g16_t[:], in_=ng30g16[:])

            D = cpool.tile([128, 256], F32, tag="D")
            S = cpool.tile([128, 256], F32, tag="S")
            stagD = cpool.tile([16, N], F32, tag="stagD")
            stagS = cpool.tile([16, N], F32, tag="stagS")

            PtD = ppool.tile([16, N], F32, name="PtD", tag="PD", bufs=1)
            PtS = ppool.tile([16, N], F32, name="PtS", tag="PS", bufs=1)
            for g in range(NG):
                raws, sqs = [], []
                for q in range(KQ):
                    raw = spool.tile([128, GRP * N], F32R, name=f"raw{q}_{g}",
                                     tag=f"raw{q}", bufs=2)
                    eng = nc.sync if q == 0 else nc.scalar
                    if g == 0:
                        # split so the first jobs' matmuls start sooner
                        eng.dma_start(out=raw[:, 0:N], in_=keysT[q, :, 0, :])
                        eng.dma_start(out=raw[:, N:GRP * N],
                                      in_=keysT[q, :, 1:GRP, :])
                    else:
                        eng.dma_start(out=raw[:],
                                      in_=keysT[q, :, GRP * g:GRP * (g + 1), :])
                    sq = spool.tile([128, GRP * N], F32R, name=f"sq{q}_{g}",
                                    tag=f"sq{q}", bufs=2)
                    if q == 0:
                        nc.scalar.square(sq[:], raw[:])
                    else:
                        nc.vector.tensor_tensor(sq[:], raw[:], raw[:],
                                                Alu.mult)
                    raws.append(raw)
                    sqs.append(sq)
                for j in range(GRP):
                    s = GRP * g + j
                    kinds = [0, 1]
                    if g == NG - 1 and j == GRP - 1:
                        kinds = [1, 0]  # sumsq first: S gates the tail
                    for kind in kinds:
                        for q in range(KQ):
                            if kind == 0:
                                lhsT = kvAll[:, (s * KQ + q) * 16:
                                             (s * KQ + q) * 16 + 16]
                                rhs_t = raws[q]
                                dst = PtD
                            else:
                                lhsT = oneAll[:, s * 16:s * 16 + 16]
                                rhs_t = sqs[q]
                                dst = PtS
                            first = (g == 0 and j == 0 and q == 0)
                            last = (g == NG - 1 and j == GRP - 1
                                    and q == KQ - 1)
                            for t_ in range(NT):
                                nc.tensor.matmul(
                                    dst[0:16, 512 * t_:512 * (t_ + 1)],
                                    lhsT,
                                    rhs_t[:, j * N + 512 * t_:
                                          j * N + 512 * (t_ + 1)],
                                    start=first, stop=last)
                        if g == NG - 1 and j == GRP - 1 and q == KQ - 1:
                            # stage + reshape as soon as each PSUM tile's
                            # accumulation closes (S closes first: the last
                            # group runs sumsq before dots)
                            if kind == 1:
                                nc.scalar.copy(stagS[:], PtS[0:16, :])
                                nc.sync.dma_start(out=S[:], in_=stagS[:])
                            else:
                                nc.vector.tensor_copy(stagD[:], PtD[0:16, :])
                                nc.scalar.dma_start(out=D[:], in_=stagD[:])

            # ---- tail on [128, 256]: partition p = 8*sample + chunk ----
            lnS = cpool.tile([128, 256], F32, tag="t1", name="lnS")
            nc.scalar.activation(lnS[:], S[:], Act.Ln)
            # rsb = exp(-0.5*lnS + ln(beta/qn)) = beta/(qn*sqrt(S))
            rsb = cpool.tile([128, 256], F32, tag="t2", name="rsb")
            nc.scalar.activation(rsb[:], lnS[:], Act.Exp, scale=-0.5,
                                 bias=lnsb_t[:])
            lg = cpool.tile([128, 256], F32, tag="t1", name="lg")
            nc.vector.tensor_tensor(lg[:], D[:], rsb[:], Alu.mult)
            lg30 = cpool.tile([128, 256], F32, tag="t3", name="lg30")
            nc.vector.tensor_scalar(lg30[:], lg[:], SHIFT, None, Alu.add)

            # unnormalized weights, overlapped with the top-k search on DVE
            ng30g_b = cpool.tile([128, 1], F32, tag="ng30gb")
            nc.vector.tensor_scalar(ng30g_b[:], gam_t[:], -SHIFT, None,
                                    Alu.mult)
            ew0 = cpool.tile([128, 256], F32, tag="ew0")
            nc.scalar.activation(ew0[:], lg30[:], Act.Exp, scale=gam_t[:],
                                 bias=ng30g_b[:])
            work = cpool.tile([128, 256], F32, tag="t2", name="work")
            nc.vector.tensor_copy(work[:], lg30[:])
            cand = cpool.tile([128, 32], F32, tag="cand")
            # [128, 32] -> [16, 256] flat-order reshape: per-sample merge.
            # candT viewed as [16][c=8][32]; round r fills cols 8r..8r+8 of
            # each 32-block, DMA'd per round to overlap with the next round.
            candT = cpool.tile([16, 8, 32], F32, tag="candT")
            for r in range(4):
                nc.vector.max(cand[:, 8 * r:8 * r + 8], work[:])
                if r < 3:
                    nc.vector.match_replace(work[:], cand[:, 8 * r:8 * r + 8],
                                            work[:], 0.0)
                eng = nc.sync if r % 2 == 0 else nc.scalar
                eng.dma_start(out=candT[:, :, 8 * r:8 * r + 8],
                              in_=cand[:, 8 * r:8 * r + 8])
            m32 = cpool.tile([16, 32], F32, tag="m32")
            for r in range(4):
                nc.vector.max(m32[:, 8 * r:8 * r + 8], candT[:])
                if r < 3:
                    nc.vector.match_replace(candT[:], m32[:, 8 * r:8 * r + 8],
                                            candT[:], 0.0)
            # m32 rows now hold each sample's global top-32 of lg30.
            # zsum = sum exp(gamma*(m32-30)) over the 32 kept entries.
            ez = cpool.tile([16, 32], F32, tag="ez")
            zsum = cpool.tile([16, 1], F32, tag="zsum")
            nc.scalar.activation(ez[:], m32[:], Act.Exp, scale=gam16_t[:],
                                 bias=ng30g16_t[:], accum_out=zsum[:])
            zr = cpool.tile([16, 1], F32, tag="zr")
            nc.vector.reciprocal(zr[:], zsum[:])
            # pack (t32, 1/zsum) pairs and broadcast [16,16] -> [128,2]
            # (partition p = 8s+c gets sample s's pair).
            pair = cpool.tile([16, 16], F32, tag="pair")
            nc.vector.tensor_copy(pair[:, 0:1], m32[:, 31:32])
            nc.vector.tensor_copy(pair[:, 1:2], zr[:])
            nc.vector.tensor_copy(pair[:, 2:4], pair[:, 0:2])
            nc.vector.tensor_copy(pair[:, 4:8], pair[:, 0:4])
            nc.vector.tensor_copy(pair[:, 8:16], pair[:, 0:8])
            pb = cpool.tile([128, 2], F32, tag="pb")
            nc.sync.dma_start(out=pb[:], in_=pair[:])
            # w = (lg30 >= t32) * ew0 / zsum
            wm = cpool.tile([128, 256], F32, tag="t2", name="wm")
            nc.vector.scalar_tensor_tensor(
                wm[:], lg30[:], pb[:, 0:1], ew0[:],
                op0=Alu.is_ge, op1=Alu.mult)
            w = cpool.tile([128, 256], F32, tag="t1", name="w")
            nc.vector.tensor_scalar(w[:], wm[:], pb[:, 1:2], None, Alu.mult)
            nc.sync.dma_start(out=out[:], in_=w[:])
    return nc


def shard_inputs(k, beta, gamma, keys):
    k = np.ascontiguousarray(k, dtype=np.float32)
    beta = np.ascontiguousarray(beta, dtype=np.float32).reshape(B)
    gamma = np.ascontiguousarray(gamma, dtype=np.float32).reshape(B)
    keys = np.asarray(keys, dtype=np.float32)
    in_maps = []
    for c in range(M):
        sl = slice(c * BPC, (c + 1) * BPC)
        kc = k[sl]                                            # [BPC, K]
        keysTc = np.ascontiguousarray(
            keys[sl].transpose(2, 0, 1)).reshape(KQ, 128, BPC, N)
        # kvTd[p, s, q, c] = kvec_s[128q+p] if c == s else 0
        kvTd_c = np.zeros((128, BPC, KQ, 16), np.float32)
        for s in range(BPC):
            kvTd_c[:, s, :, s] = kc[s].reshape(KQ, 128).T
        # onesT[p, s, c] = 1.0 if c == s else 0
        onesT_c = np.zeros((128, BPC, 16), np.float32)
        for s in range(BPC):
            onesT_c[:, s, s] = 1.0
        qn = np.maximum(np.linalg.norm(kc.astype(np.float64), axis=-1), EPS)
        lnsb_s = np.log(beta[sl].astype(np.float64) / qn).astype(np.float32)
        gam_s = gamma[sl]
        # partition p = 8*s + c  ->  per-partition sample index p // 8
        rep = np.repeat(np.arange(BPC), 8)
        in_maps.append({
            "keysT": keysTc,
            "kvTd": kvTd_c.reshape(128, BPC * KQ * 16),
            "onesT": onesT_c.reshape(128, BPC * 16),
            "lnsb": lnsb_s[rep].reshape(128, 1),
            "gam": gam_s[rep].reshape(128, 1).astype(np.float32),
            "gam16": gam_s.reshape(16, 1).astype(np.float32),
            "ng30g16": (-SHIFT * gam_s).reshape(16, 1).astype(np.float32),
        })
    return in_maps


_NC_CACHE = None


def kernel(k=None, beta=None, gamma=None, keys=None, candidates=None,
           **_ignored):
    assert int(candidates) == CAND, \
        f"kernel hardcoded for candidates=32, got {candidates}"
    assert keys.shape == (B, N, K), keys.shape
    global _NC_CACHE
    if _NC_CACHE is None:
        _NC_CACHE = build_nc()
    in_maps = shard_inputs(k, beta, gamma, keys)
    res = run_bass_kernel_spmd(_NC_CACHE, in_maps, list(range(M))).results
    return np.concatenate(
        [res[c]["out"].reshape(BPC, N) for c in range(M)], axis=0)


# revision 18
# speedup vs baseline: 1.0599x; 1.0599x over previous
"""Trainium2 Bass kernel for nn_GCLMemory (content-addressed memory read weights).

Full computation per batch sample b:
    dots[n]  = <keys[b,n,:], k[b,:]>
    cos[n]   = dots[n] / (max(||keys[b,n]||,eps) * max(||k[b]||,eps))
    wc       = softmax(beta[b] * cos)
    top-32 mask, renormalize, w = wc**gamma[b], renormalize.

Tail identity: the intermediate renormalizations cancel, so
    w = exp(gamma*logits) / sum  over the top-32 logits positions,
    logits = beta*cos.  Non-top leakage (1e-16 factor) is below fp32
    noise after **gamma, so it is dropped.

Sharding: data-parallel over batch. 8 cores x 16 samples.

Device-side layout (per core):
  - Host pre-transposes + casts keys to bf16: keysT [KQ=2, 128, 16, 2048]
    (K on partitions).  bf16 halves HBM traffic and runs the PE at
    1 cycle/row instead of fp32's 4.
  - lhsT kvT [KQ, 128, 32] bf16: col s = kvec of sample s, col 16 = ones.
    Per (sample, kind) job the PE streams 512-col slices of the raw
    (dots) or squared (sumsq) stream tile; useful output rows are s
    (dots) and 16 (sumsq).  4 jobs pack one [128, 2048] PSUM tile at
    partition bases 0/32/64/96.
  - PSUM tiles are bulk-copied to SBUF on ACT; per-job useful rows are
    extracted by DMA into D/S in a [128, 256] layout with partition
    p = 8*s + c (c = 256-col chunk of n), so the whole tail runs with
    128 active partitions (DVE/ACT cost scales with free size only).
  - Tail: rsb = exp(-0.5*ln(S) + ln(beta/||k||)); lg30 = D*rsb + 30
    (the +30 shift keeps all values positive for the match_replace
    top-k trick and is undone by the Exp bias); per-partition top-32
    via 4x max8 + match_replace; cross-chunk merge after a [128,32] ->
    [16,256] DMA reshape; threshold mask via is_ge; w = Exp(gamma*x -
    30*gamma) with accumulate; renorm via DMA-reshaped partial sums.
"""

import sys

import numpy as np

sys.path.insert(0, "/opt/trn_rl_repo")

import concourse.bass as bass
import concourse.mybir as mybir
from concourse.bass_utils import run_bass_kernel_spmd
from concourse.tile import TileContext

F32 = mybir.dt.float32
F32R = mybir.dt.float32r
BF16 = mybir.dt.bfloat16
Alu = mybir.AluOpType
Act = mybir.ActivationFunctionType
AxL = mybir.AxisListType

# ---------------------------------------------------------------------------
# This container's walrus build only accepts a single sem-wait command per
# instruction ("Too many sync wait commands" in CoreV3GenImpl otherwise), but
# Tile's exit drain aggregates one wait per busy processor. Split any
# multi-wait instruction into preceding single-wait Drains on the same engine.
# ---------------------------------------------------------------------------
_WAIT_LIMIT = 1


def _split_multi_waits(bir_bytes: bytes, limit: int = _WAIT_LIMIT) -> bytes:
    import orjson
    d = orjson.loads(bir_bytes)
    n_split = 0
    for fn in d.get("functions", []):
        for bb in fn.get("blocks", []):
            out = []
            for inst in bb.get("instructions", []):
                si = inst.get("sync_info") or {}
                waits = si.get("on_wait") or []
                if len(waits) > limit:
                    n_split += 1
                    chunks = [waits[i:i + limit]
                              for i in range(0, len(waits), limit)]
                    for j, ch in enumerate(chunks[:-1]):
                        carrier = {
                            "engine": inst["engine"],
                            "ins": [],
                            "is_reset_sema": False,
                            "name": f"{inst['name']}__w{j}",
                            "opcode": "Drain",
                            "outs": [],
                            "sync_info": {"on_update": [], "on_wait": ch},
                        }
                        if "debug" in inst:
                            carrier["debug"] = inst["debug"]
                        out.append(carrier)
                    si["on_wait"] = chunks[-1]
                out.append(inst)
            bb["instructions"] = out
    return orjson.dumps(d)


def _install_wait_split_hook():
    from concourse import bass2jax
    orig = bass2jax.compile_bir_kernel
    if getattr(orig, "_wait_split_wrapped", False):
        return

    def wrapped(bir_bytes, *args, **kwargs):
        return orig(_split_multi_waits(bir_bytes), *args, **kwargs)

    wrapped._wait_split_wrapped = True
    bass2jax.compile_bir_kernel = wrapped


_install_wait_split_hook()

B, N, K = 128, 2048, 256
M = 8            # cores
BPC = B // M     # samples per core
KQ = K // 128    # contraction chunks
NT = N // 512    # psum column tiles per sample
CAND = 32
GRP = 2          # samples per stream tile (16KB fp32 DMA lines)
NG = BPC // GRP  # stream groups
EPS = 1e-8
SHIFT = 30.0     # logits shift: keeps lg30 > 0 for the match_replace trick


def build_nc():
    nc = bass.Bass()
    keysT = nc.declare_dram_parameter("keysT", [KQ, 128, BPC, N], F32R,
                                      isOutput=False)
    kvTd = nc.declare_dram_parameter("kvTd", [128, BPC * KQ * 16], F32R,
                                     isOutput=False)
    onesT = nc.declare_dram_parameter("onesT", [128, BPC * 16], F32R,
                                      isOutput=False)
    lnsb = nc.declare_dram_parameter("lnsb", [128, 1], F32, isOutput=False)
    gam = nc.declare_dram_parameter("gam", [128, 1], F32, isOutput=False)
    gam16 = nc.declare_dram_parameter("gam16", [16, 1], F32, isOutput=False)
    ng30g16 = nc.declare_dram_parameter("ng30g16", [16, 1], F32,
                                        isOutput=False)
    out = nc.declare_dram_parameter("out", [128, N // 8], F32, isOutput=True)

    with TileContext(nc) as tc:
        with (
            tc.tile_pool(name="const", bufs=1) as cpool,
            tc.tile_pool(name="stream", bufs=3) as spool,
            tc.tile_pool(name="psum", bufs=1, space="PSUM") as ppool,
        ):
            # lhsT banks: per (sample s, q) a [128,32] fp32r tile with
            # kvec_s chunk q at col s (zeros elsewhere); per sample a ones
            # tile with 1.0 at col 16+s.  fp32r matmuls must write PSUM
            # partition base 0, so all 32 jobs accumulate into ONE shared
            # [32, 2048] PSUM tile: rows 0-15 collect dots (col s -> row s),
            # rows 16-31 collect sumsq; zero lhsT columns contribute 0.
            kvAll = cpool.tile([128, BPC * KQ * 16], F32R, tag="kvAll")
            nc.scalar.dma_start(out=kvAll[:], in_=kvTd[:])
            oneAll = cpool.tile([128, BPC * 16], F32R, tag="oneAll")
            nc.scalar.dma_start(out=oneAll[:], in_=onesT[:])
            lnsb_t = cpool.tile([128, 1], F32, tag="lnsb")
            nc.scalar.dma_start(out=lnsb_t[:], in_=lnsb[:])
            gam_t = cpool.tile([128, 1], F32, tag="gam")
            nc.scalar.dma_start(out=gam_t[:], in_=gam[:])
            gam16_t = cpool.tile([16, 1], F32, tag="gam16")
            nc.scalar.dma_start(out=gam16_t[:], in_=gam16[:])
            ng30g16_t = cpool.tile([16, 1], F32, tag="ng30g16")
            nc.scalar.dma_start(out=ng30g16_t[:], in_=ng30g16[:])

            D = cpool.tile([128, 256], F32, tag="D")
            S = cpool.tile([128, 256], F32, tag="S")
            stagD = cpool.tile([16, N], F32, tag="stagD")
            stagS = cpool.tile([16, N], F32, tag="stagS")

            PtD = ppool.tile([16, N], F32, name="PtD", tag="PD", bufs=1)
            PtS = ppool.tile([16, N], F32, name="PtS", tag="PS", bufs=1)
            for g in range(NG):
                raws, sqs = [], []
                for q in range(KQ):
                    raw = spool.tile([128, GRP * N], F32R, name=f"raw{q}_{g}",
                                     tag=f"raw{q}", bufs=2)
                    eng = nc.sync if q == 0 else nc.scalar
                    if g == 0:
                        # split so the first jobs' matmuls start sooner
                        eng.dma_start(out=raw[:, 0:N], in_=keysT[q, :, 0, :])
                        eng.dma_start(out=raw[:, N:GRP * N],
                                      in_=keysT[q, :, 1:GRP, :])
                    else:
                        eng.dma_start(out=raw[:],
                                      in_=keysT[q, :, GRP * g:GRP * (g + 1), :])
                    sq = spool.tile([128, GRP * N], F32R, name=f"sq{q}_{g}",
                                    tag=f"sq{q}", bufs=2)
                    if q == 0:
                        nc.scalar.square(sq[:], raw[:])
                    else:
                        nc.vector.tensor_tensor(sq[:], raw[:], raw[:],
                                                Alu.mult)
                    raws.append(raw)
                    sqs.append(sq)
                for j in range(GRP):
                    s = GRP * g + j
                    kinds = [0, 1]
                    if g == NG - 1 and j == GRP - 1:
                        kinds = [1, 0]  # sumsq first: S gates the tail
                    for kind in kinds:
                        for q in range(KQ):
                            if kind == 0:
                                lhsT = kvAll[:, (s * KQ + q) * 16:
                                             (s * KQ + q) * 16 + 16]
                                rhs_t = raws[q]
                                dst = PtD
                            else:
                                lhsT = oneAll[:, s * 16:s * 16 + 16]
                                rhs_t = sqs[q]
                                dst = PtS
                            first = (g == 0 and j == 0 and q == 0)
                            last = (g == NG - 1 and j == GRP - 1
                                    and q == KQ - 1)
                            for t_ in range(NT):
                                nc.tensor.matmul(
                                    dst[0:16, 512 * t_:512 * (t_ + 1)],
                                    lhsT,
                                    rhs_t[:, j * N + 512 * t_:
                                          j * N + 512 * (t_ + 1)],
                                    start=first, stop=last)

            nc.scalar.copy(stagD[:], PtD[0:16, :])
            nc.vector.tensor_copy(stagS[:], PtS[0:16, :])
            nc.scalar.dma_start(out=D[:], in_=stagD[:])
            nc.sync.dma_start(out=S[:], in_=stagS[:])

            # ---- tail on [128, 256]: partition p = 8*sample + chunk ----
            lnS = cpool.tile([128, 256], F32, tag="t1", name="lnS")
            nc.scalar.activation(lnS[:], S[:], Act.Ln)
            # rsb = exp(-0.5*lnS + ln(beta/qn)) = beta/(qn*sqrt(S))
            rsb = cpool.tile([128, 256], F32, tag="t2", name="rsb")
            nc.scalar.activation(rsb[:], lnS[:], Act.Exp, scale=-0.5,
                                 bias=lnsb_t[:])
            lg = cpool.tile([128, 256], F32, tag="t1", name="lg")
            nc.vector.tensor_tensor(lg[:], D[:], rsb[:], Alu.mult)
            lg30 = cpool.tile([128, 256], F32, tag="t3", name="lg30")
            nc.vector.tensor_scalar(lg30[:], lg[:], SHIFT, None, Alu.add)

            # unnormalized weights, overlapped with the top-k search on DVE
            ng30g_b = cpool.tile([128, 1], F32, tag="ng30gb")
            nc.vector.tensor_scalar(ng30g_b[:], gam_t[:], -SHIFT, None,
                                    Alu.mult)
            ew0 = cpool.tile([128, 256], F32, tag="ew0")
            nc.scalar.activation(ew0[:], lg30[:], Act.Exp, scale=gam_t[:],
                                 bias=ng30g_b[:])
            work = cpool.tile([128, 256], F32, tag="t2", name="work")
            nc.vector.tensor_copy(work[:], lg30[:])
            cand = cpool.tile([128, 32], F32, tag="cand")
            # [128, 32] -> [16, 256] flat-order reshape: per-sample merge.
            # candT viewed as [16][c=8][32]; round r fills cols 8r..8r+8 of
            # each 32-block, DMA'd per round to overlap with the next round.
            candT = cpool.tile([16, 8, 32], F32, tag="candT")
            for r in range(4):
                nc.vector.max(cand[:, 8 * r:8 * r + 8], work[:])
                if r < 3:
                    nc.vector.match_replace(work[:], cand[:, 8 * r:8 * r + 8],
                                            work[:], 0.0)
                eng = nc.sync if r % 2 == 0 else nc.scalar
                eng.dma_start(out=candT[:, :, 8 * r:8 * r + 8],
                              in_=cand[:, 8 * r:8 * r + 8])
            m32 = cpool.tile([16, 32], F32, tag="m32")
            for r in range(4):
                nc.vector.max(m32[:, 8 * r:8 * r + 8], candT[:])
                if r < 3:
                    nc.vector.match_replace(candT[:], m32[:, 8 * r:8 * r + 8],
                                            candT[:], 0.0)
            # m32 rows now hold each sample's global top-32 of lg30.
            # zsum = sum exp(gamma*(m32-30)) over the 32 kept entries.
            ez = cpool.tile([16, 32], F32, tag="ez")
            zsum = cpool.tile([16, 1], F32, tag="zsum")
            nc.scalar.activation(ez[:], m32[:], Act.Exp, scale=gam16_t[:],
                                 bias=ng30g16_t[:], accum_out=zsum[:])
            zr = cpool.tile([16, 1], F32, tag="zr")
            nc.vector.reciprocal(zr[:], zsum[:])
            # pack (t32, 1/zsum) pairs and broadcast [16,16] -> [128,2]
            # (partition p = 8s+c gets sample s's pair).
            pair = cpool.tile([16, 16], F32, tag="pair")
            nc.vector.tensor_copy(pair[:, 0:1], m32[:, 31:32])
            nc.vector.tensor_copy(pair[:, 1:2], zr[:])
            nc.vector.tensor_copy(pair[:, 2:4], pair[:, 0:2])
            nc.vector.tensor_copy(pair[:, 4:8], pair[:, 0:4])
            nc.vector.tensor_copy(pair[:, 8:16], pair[:, 0:8])
            pb = cpool.tile([128, 2], F32, tag="pb")
            nc.sync.dma_start(out=pb[:], in_=pair[:])
            # w = (lg30 >= t32) * ew0 / zsum
            wm = cpool.tile([128, 256], F32, tag="t2", name="wm")
            nc.vector.scalar_tensor_tensor(
                wm[:], lg30[:], pb[:, 0:1], ew0[:],
                op0=Alu.is_ge, op1=Alu.mult)
            w = cpool.tile([128, 256], F32, tag="t1", name="w")
            nc.vector.tensor_scalar(w[:], wm[:], pb[:, 1:2], None, Alu.mult)
            nc.sync.dma_start(out=out[:], in_=w[:])
    return nc


def shard_inputs(k, beta, gamma, keys):
    k = np.ascontiguousarray(k, dtype=np.float32)
    beta = np.ascontiguousarray(beta, dtype=np.float32).reshape(B)
    gamma = np.ascontiguousarray(gamma, dtype=np.float32).reshape(B)
    keys = np.asarray(keys, dtype=np.float32)
    in_maps = []
    for c in range(M):
        sl = slice(c * BPC, (c + 1) * BPC)
        kc = k[sl]                                            # [BPC, K]
        keysTc = np.ascontiguousarray(
            keys[sl].transpose(2, 0, 1)).reshape(KQ, 128, BPC, N)
        # kvTd[p, s, q, c] = kvec_s[128q+p] if c == s else 0
        kvTd_c = np.zeros((128, BPC, KQ, 16), np.float32)
        for s in range(BPC):
            kvTd_c[:, s, :, s] = kc[s].reshape(KQ, 128).T
        # onesT[p, s, c] = 1.0 if c == s else 0
        onesT_c = np.zeros((128, BPC, 16), np.float32)
        for s in range(BPC):
            onesT_c[:, s, s] = 1.0
        qn = np.maximum(np.linalg.norm(kc.astype(np.float64), axis=-1), EPS)
        lnsb_s = np.log(beta[sl].astype(np.float64) / qn).astype(np.float32)
        gam_s = gamma[sl]
        # partition p = 8*s + c  ->  per-partition sample index p // 8
        rep = np.repeat(np.arange(BPC), 8)
        in_maps.append({
            "keysT": keysTc,
            "kvTd": kvTd_c.reshape(128, BPC * KQ * 16),
            "onesT": onesT_c.reshape(128, BPC * 16),
            "lnsb": lnsb_s[rep].reshape(128, 1),
            "gam": gam_s[rep].reshape(128, 1).astype(np.float32),
            "gam16": gam_s.reshape(16, 1).astype(np.float32),
            "ng30g16": (-SHIFT * gam_s).reshape(16, 1).astype(np.float32),
        })
    return in_maps


_NC_CACHE = None


def kernel(k=None, beta=None, gamma=None, keys=None, candidates=None,
           **_ignored):
    assert int(candidates) == CAND, \
        f"kernel hardcoded for candidates=32, got {candidates}"
    assert keys.shape == (B, N, K), keys.shape
    global _NC_CACHE
    if _NC_CACHE is None:
        _NC_CACHE = build_nc()
    in_maps = shard_inputs(k, beta, gamma, keys)
    res = run_bass_kernel_spmd(_NC_CACHE, in_maps, list(range(M))).results
    return np.concatenate(
        [res[c]["out"].reshape(BPC, N) for c in range(M)], axis=0)


# revision 19
# speedup vs baseline: 1.1807x; 1.1140x over previous
"""Trainium2 Bass kernel for nn_GCLMemory (content-addressed memory read weights).

Full computation per batch sample b:
    dots[n]  = <keys[b,n,:], k[b,:]>
    cos[n]   = dots[n] / (max(||keys[b,n]||,eps) * max(||k[b]||,eps))
    wc       = softmax(beta[b] * cos)
    top-32 mask, renormalize, w = wc**gamma[b], renormalize.

Tail identity: the intermediate renormalizations cancel, so
    w = exp(gamma*logits) / sum  over the top-32 logits positions,
    logits = beta*cos.  Non-top leakage (1e-16 factor) is below fp32
    noise after **gamma, so it is dropped.

Sharding: data-parallel over batch. 8 cores x 16 samples.

Device-side layout (per core):
  - Host pre-transposes + casts keys to bf16: keysT [KQ=2, 128, 16, 2048]
    (K on partitions).  bf16 halves HBM traffic and runs the PE at
    1 cycle/row instead of fp32's 4.
  - lhsT kvT [KQ, 128, 32] bf16: col s = kvec of sample s, col 16 = ones.
    Per (sample, kind) job the PE streams 512-col slices of the raw
    (dots) or squared (sumsq) stream tile; useful output rows are s
    (dots) and 16 (sumsq).  4 jobs pack one [128, 2048] PSUM tile at
    partition bases 0/32/64/96.
  - PSUM tiles are bulk-copied to SBUF on ACT; per-job useful rows are
    extracted by DMA into D/S in a [128, 256] layout with partition
    p = 8*s + c (c = 256-col chunk of n), so the whole tail runs with
    128 active partitions (DVE/ACT cost scales with free size only).
  - Tail: rsb = exp(-0.5*ln(S) + ln(beta/||k||)); lg30 = D*rsb + 30
    (the +30 shift keeps all values positive for the match_replace
    top-k trick and is undone by the Exp bias); per-partition top-32
    via 4x max8 + match_replace; cross-chunk merge after a [128,32] ->
    [16,256] DMA reshape; threshold mask via is_ge; w = Exp(gamma*x -
    30*gamma) with accumulate; renorm via DMA-reshaped partial sums.
"""

import sys

import numpy as np

sys.path.insert(0, "/opt/trn_rl_repo")

import concourse.bass as bass
import concourse.mybir as mybir
from concourse.bass_utils import run_bass_kernel_spmd
from concourse.tile import TileContext

F32 = mybir.dt.float32
F32R = mybir.dt.float32r
BF16 = mybir.dt.bfloat16
Alu = mybir.AluOpType
Act = mybir.ActivationFunctionType
AxL = mybir.AxisListType

# ---------------------------------------------------------------------------
# This container's walrus build only accepts a single sem-wait command per
# instruction ("Too many sync wait commands" in CoreV3GenImpl otherwise), but
# Tile's exit drain aggregates one wait per busy processor. Split any
# multi-wait instruction into preceding single-wait Drains on the same engine.
# ---------------------------------------------------------------------------
_WAIT_LIMIT = 1


def _split_multi_waits(bir_bytes: bytes, limit: int = _WAIT_LIMIT) -> bytes:
    import orjson
    d = orjson.loads(bir_bytes)
    n_split = 0
    for fn in d.get("functions", []):
        for bb in fn.get("blocks", []):
            out = []
            for inst in bb.get("instructions", []):
                si = inst.get("sync_info") or {}
                waits = si.get("on_wait") or []
                if len(waits) > limit:
                    n_split += 1
                    chunks = [waits[i:i + limit]
                              for i in range(0, len(waits), limit)]
                    for j, ch in enumerate(chunks[:-1]):
                        carrier = {
                            "engine": inst["engine"],
                            "ins": [],
                            "is_reset_sema": False,
                            "name": f"{inst['name']}__w{j}",
                            "opcode": "Drain",
                            "outs": [],
                            "sync_info": {"on_update": [], "on_wait": ch},
                        }
                        if "debug" in inst:
                            carrier["debug"] = inst["debug"]
                        out.append(carrier)
                    si["on_wait"] = chunks[-1]
                out.append(inst)
            bb["instructions"] = out
    return orjson.dumps(d)


def _install_wait_split_hook():
    from concourse import bass2jax
    orig = bass2jax.compile_bir_kernel
    if getattr(orig, "_wait_split_wrapped", False):
        return

    def wrapped(bir_bytes, *args, **kwargs):
        return orig(_split_multi_waits(bir_bytes), *args, **kwargs)

    wrapped._wait_split_wrapped = True
    bass2jax.compile_bir_kernel = wrapped


_install_wait_split_hook()

B, N, K = 128, 2048, 256
M = 8            # cores
BPC = B // M     # samples per core
KQ = K // 128    # contraction chunks
NT = N // 512    # psum column tiles per sample
CAND = 32
GRP = 2          # samples per stream tile (16KB fp32 DMA lines)
NG = BPC // GRP  # stream groups
EPS = 1e-8
SHIFT = 30.0     # logits shift: keeps lg30 > 0 for the match_replace trick


def build_nc():
    nc = bass.Bass()
    keysT = nc.declare_dram_parameter("keysT", [KQ, 128, BPC, N], F32R,
                                      isOutput=False)
    kvTd = nc.declare_dram_parameter("kvTd", [128, BPC * KQ * 16], F32R,
                                     isOutput=False)
    onesT = nc.declare_dram_parameter("onesT", [128, BPC * 16], F32R,
                                      isOutput=False)
    lnsb = nc.declare_dram_parameter("lnsb", [128, 1], F32, isOutput=False)
    gam = nc.declare_dram_parameter("gam", [128, 1], F32, isOutput=False)
    gam16 = nc.declare_dram_parameter("gam16", [16, 1], F32, isOutput=False)
    ng30g16 = nc.declare_dram_parameter("ng30g16", [16, 1], F32,
                                        isOutput=False)
    out = nc.declare_dram_parameter("out", [128, N // 8], F32, isOutput=True)

    with TileContext(nc) as tc:
        with (
            tc.tile_pool(name="const", bufs=1) as cpool,
            tc.tile_pool(name="stream", bufs=3) as spool,
            tc.tile_pool(name="psum", bufs=1, space="PSUM") as ppool,
        ):
            # lhsT banks: per (sample s, q) a [128,32] fp32r tile with
            # kvec_s chunk q at col s (zeros elsewhere); per sample a ones
            # tile with 1.0 at col 16+s.  fp32r matmuls must write PSUM
            # partition base 0, so all 32 jobs accumulate into ONE shared
            # [32, 2048] PSUM tile: rows 0-15 collect dots (col s -> row s),
            # rows 16-31 collect sumsq; zero lhsT columns contribute 0.
            kvAll = cpool.tile([128, BPC * KQ * 16], F32R, tag="kvAll")
            nc.scalar.dma_start(out=kvAll[:], in_=kvTd[:])
            oneAll = cpool.tile([128, BPC * 16], F32R, tag="oneAll")
            nc.scalar.dma_start(out=oneAll[:], in_=onesT[:])
            lnsb_t = cpool.tile([128, 1], F32, tag="lnsb")
            nc.scalar.dma_start(out=lnsb_t[:], in_=lnsb[:])
            gam_t = cpool.tile([128, 1], F32, tag="gam")
            nc.scalar.dma_start(out=gam_t[:], in_=gam[:])
            gam16_t = cpool.tile([16, 1], F32, tag="gam16")
            nc.scalar.dma_start(out=gam16_t[:], in_=gam16[:])
            ng30g16_t = cpool.tile([16, 1], F32, tag="ng30g16")
            nc.scalar.dma_start(out=ng30g16_t[:], in_=ng30g16[:])

            D = cpool.tile([128, 256], F32, tag="D")
            S = cpool.tile([128, 256], F32, tag="S")

            PtD = ppool.tile([16, N], F32, name="PtD", tag="PD", bufs=1)
            PtS = ppool.tile([16, N], F32, name="PtS", tag="PS", bufs=1)
            for g in range(NG):
                raws, sqs = [], []
                for q in range(KQ):
                    raw = spool.tile([128, GRP * N], F32R, name=f"raw{q}_{g}",
                                     tag=f"raw{q}", bufs=2)
                    eng = nc.sync if q == 0 else nc.scalar
                    if g == 0:
                        # split so the first jobs' matmuls start sooner
                        eng.dma_start(out=raw[:, 0:N], in_=keysT[q, :, 0, :])
                        eng.dma_start(out=raw[:, N:GRP * N],
                                      in_=keysT[q, :, 1:GRP, :])
                    else:
                        eng.dma_start(out=raw[:],
                                      in_=keysT[q, :, GRP * g:GRP * (g + 1), :])
                    sq = spool.tile([128, GRP * N], F32R, name=f"sq{q}_{g}",
                                    tag=f"sq{q}", bufs=2)
                    if q == 0:
                        nc.scalar.square(sq[:], raw[:])
                    else:
                        nc.vector.tensor_tensor(sq[:], raw[:], raw[:],
                                                Alu.mult)
                    raws.append(raw)
                    sqs.append(sq)
                for j in range(GRP):
                    s = GRP * g + j
                    kinds = [0, 1]
                    if g == NG - 1 and j == GRP - 1:
                        kinds = [1, 0]  # sumsq first: S gates the tail
                    for kind in kinds:
                        for q in range(KQ):
                            if kind == 0:
                                lhsT = kvAll[:, (s * KQ + q) * 16:
                                             (s * KQ + q) * 16 + 16]
                                rhs_t = raws[q]
                                dst = PtD
                            else:
                                lhsT = oneAll[:, s * 16:s * 16 + 16]
                                rhs_t = sqs[q]
                                dst = PtS
                            first = (g == 0 and j == 0 and q == 0)
                            last = (g == NG - 1 and j == GRP - 1
                                    and q == KQ - 1)
                            for t_ in range(NT):
                                nc.tensor.matmul(
                                    dst[0:16, 512 * t_:512 * (t_ + 1)],
                                    lhsT,
                                    rhs_t[:, j * N + 512 * t_:
                                          j * N + 512 * (t_ + 1)],
                                    start=first, stop=last)

            stagD = cpool.tile([16, N], F32, tag="stagD")
            stagS = cpool.tile([16, N], F32, tag="stagS")
            nc.scalar.copy(stagD[:], PtD[0:16, :])
            nc.vector.tensor_copy(stagS[:], PtS[0:16, :])
            nc.scalar.dma_start(out=D[:], in_=stagD[:])
            nc.sync.dma_start(out=S[:], in_=stagS[:])

            # ---- tail on [128, 256]: partition p = 8*sample + chunk ----
            lnS = cpool.tile([128, 256], F32, tag="t1", name="lnS")
            nc.scalar.activation(lnS[:], S[:], Act.Ln)
            # rsb = exp(-0.5*lnS + ln(beta/qn)) = beta/(qn*sqrt(S))
            rsb = cpool.tile([128, 256], F32, tag="t2", name="rsb")
            nc.scalar.activation(rsb[:], lnS[:], Act.Exp, scale=-0.5,
                                 bias=lnsb_t[:])
            lg = cpool.tile([128, 256], F32, tag="t1", name="lg")
            nc.vector.tensor_tensor(lg[:], D[:], rsb[:], Alu.mult)
            lg30 = cpool.tile([128, 256], F32, tag="t3", name="lg30")
            nc.vector.tensor_scalar(lg30[:], lg[:], SHIFT, None, Alu.add)

            # unnormalized weights, overlapped with the top-k search on DVE
            ng30g_b = cpool.tile([128, 1], F32, tag="ng30gb")
            nc.vector.tensor_scalar(ng30g_b[:], gam_t[:], -SHIFT, None,
                                    Alu.mult)
            ew0 = cpool.tile([128, 256], F32, tag="ew0")
            nc.scalar.activation(ew0[:], lg30[:], Act.Exp, scale=gam_t[:],
                                 bias=ng30g_b[:])
            work = cpool.tile([128, 256], F32, tag="t2", name="work")
            nc.vector.tensor_copy(work[:], lg30[:])
            cand = cpool.tile([128, 32], F32, tag="cand")
            # [128, 32] -> [16, 256] flat-order reshape: per-sample merge.
            # candT viewed as [16][c=8][32]; round r fills cols 8r..8r+8 of
            # each 32-block, DMA'd per round to overlap with the next round.
            candT = cpool.tile([16, 8, 32], F32, tag="candT")
            for r in range(4):
                nc.vector.max(cand[:, 8 * r:8 * r + 8], work[:])
                if r < 3:
                    nc.vector.match_replace(work[:], cand[:, 8 * r:8 * r + 8],
                                            work[:], 0.0)
                eng = nc.sync if r % 2 == 0 else nc.scalar
                eng.dma_start(out=candT[:, :, 8 * r:8 * r + 8],
                              in_=cand[:, 8 * r:8 * r + 8])
            m32 = cpool.tile([16, 32], F32, tag="m32")
            for r in range(4):
                nc.vector.max(m32[:, 8 * r:8 * r + 8], candT[:])
                if r < 3:
                    nc.vector.match_replace(candT[:], m32[:, 8 * r:8 * r + 8],
                                            candT[:], 0.0)
            # m32 rows now hold each sample's global top-32 of lg30.
            # zsum = sum exp(gamma*(m32-30)) over the 32 kept entries.
            ez = cpool.tile([16, 32], F32, tag="ez")
            zsum = cpool.tile([16, 1], F32, tag="zsum")
            nc.scalar.activation(ez[:], m32[:], Act.Exp, scale=gam16_t[:],
                                 bias=ng30g16_t[:], accum_out=zsum[:])
            zr = cpool.tile([16, 1], F32, tag="zr")
            nc.vector.reciprocal(zr[:], zsum[:])
            # pack (t32, 1/zsum) pairs and broadcast [16,16] -> [128,2]
            # (partition p = 8s+c gets sample s's pair).
            pair = cpool.tile([16, 16], F32, tag="pair")
            nc.vector.tensor_copy(pair[:, 0:1], m32[:, 31:32])
            nc.vector.tensor_copy(pair[:, 1:2], zr[:])
            nc.vector.tensor_copy(pair[:, 2:4], pair[:, 0:2])
            nc.vector.tensor_copy(pair[:, 4:8], pair[:, 0:4])
            nc.vector.tensor_copy(pair[:, 8:16], pair[:, 0:8])
            pb = cpool.tile([128, 2], F32, tag="pb")
            nc.sync.dma_start(out=pb[:], in_=pair[:])
            # w = (lg30 >= t32) * ew0 / zsum
            wm = cpool.tile([128, 256], F32, tag="t2", name="wm")
            nc.vector.scalar_tensor_tensor(
                wm[:], lg30[:], pb[:, 0:1], ew0[:],
                op0=Alu.is_ge, op1=Alu.mult)
            w = cpool.tile([128, 256], F32, tag="t1", name="w")
            nc.vector.tensor_scalar(w[:], wm[:], pb[:, 1:2], None, Alu.mult)
            nc.sync.dma_start(out=out[:], in_=w[:])
    return nc


def shard_inputs(k, beta, gamma, keys):
    k = np.ascontiguousarray(k, dtype=np.float32)
    beta = np.ascontiguousarray(beta, dtype=np.float32).reshape(B)
    gamma = np.ascontiguousarray(gamma, dtype=np.float32).reshape(B)
    keys = np.asarray(keys, dtype=np.float32)
    in_maps = []
    for c in range(M):
        sl = slice(c * BPC, (c + 1) * BPC)
        kc = k[sl]                                            # [BPC, K]
        keysTc = np.ascontiguousarray(
            keys[sl].transpose(2, 0, 1)).reshape(KQ, 128, BPC, N)
        # kvTd[p, s, q, c] = kvec_s[128q+p] if c == s else 0
        kvTd_c = np.zeros((128, BPC, KQ, 16), np.float32)
        for s in range(BPC):
            kvTd_c[:, s, :, s] = kc[s].reshape(KQ, 128).T
        # onesT[p, s, c] = 1.0 if c == s else 0
        onesT_c = np.zeros((128, BPC, 16), np.float32)
        for s in range(BPC):
            onesT_c[:, s, s] = 1.0
        qn = np.maximum(np.linalg.norm(kc.astype(np.float64), axis=-1), EPS)
        lnsb_s = np.log(beta[sl].astype(np.float64) / qn).astype(np.float32)
        gam_s = gamma[sl]
        # partition p = 8*s + c  ->  per-partition sample index p // 8
        rep = np.repeat(np.arange(BPC), 8)
        in_maps.append({
            "keysT": keysTc,
            "kvTd": kvTd_c.reshape(128, BPC * KQ * 16),
            "onesT": onesT_c.reshape(128, BPC * 16),
            "lnsb": lnsb_s[rep].reshape(128, 1),
            "gam": gam_s[rep].reshape(128, 1).astype(np.float32),
            "gam16": gam_s.reshape(16, 1).astype(np.float32),
            "ng30g16": (-SHIFT * gam_s).reshape(16, 1).astype(np.float32),
        })
    return in_maps


_NC_CACHE = None


def kernel(k=None, beta=None, gamma=None, keys=None, candidates=None,
           **_ignored):
    assert int(candidates) == CAND, \
        f"kernel hardcoded for candidates=32, got {candidates}"
    assert keys.shape == (B, N, K), keys.shape
    global _NC_CACHE
    if _NC_CACHE is None:
        _NC_CACHE = build_nc()
    in_maps = shard_inputs(k, beta, gamma, keys)
    res = run_bass_kernel_spmd(_NC_CACHE, in_maps, list(range(M))).results
    return np.concatenate(
        [res[c]["out"].reshape(BPC, N) for c in range(M)], axis=0)
